# revision 31
# baseline (speedup 1.0000x reference)
"""FP4 (e2m1-packed) column-parallel Linear: y = x @ W^T + b on 8 NeuronCores.

Strategy
--------
- Tensor-parallel over out_features (column-parallel linear): each of the 8
  cores owns 1376 output features (padded to 1408 = 11*128), x is replicated.
- The e2m1 value set {0, +-0.5, 1, 1.5, 2, 3, 4, 6} is exactly representable
  in fp8-e4m3, which the PE consumes natively.  The host unpacks each nibble
  into one fp8 byte (pure byte-LUT, part of the sharding/layout step) and the
  device kernel is a straight fp8(W, stationary) x bf16(x, moving) matmul
  accumulating in f32 PSUM -- no on-chip dequant work at all.
- o-tiles are processed in groups of <=4 with the k-loop outer, so the PE
  consumes x chunks as they stream in; all input DMAs round-robin across both
  HWDGE engines (Sync, Scalar), ordered so every tile lands just ahead of the
  PE's consumption point.
- A short burst of junk matmuls at kernel start trips the PE HAM clock-gate
  (which needs ~3.4us of sustained activity) during the DMA-wait window, so
  real matmuls run at the warm 2.4 GHz clock from the first instruction.
- PSUM drains are split into column halves across Scalar (ACT with fused
  per-partition bias) and Vector (tensor_scalar add) so drains never
  serialize behind one engine; output DMAs alternate between the engines.

Host-side layouts (every DMA per-partition contiguous):
    xt  [128, 32*512]   bf16  xt[kk, t*512+s]     = x[s, t*128+kk] (replicated)
    wt  [11, 128, 4096] fp8   wt[ot, kk, t*128+o] = W[ot*128+o + core_off,
                                                      t*128+kk]
    bias[128, 11]       f32   bias[p, ot]         = b[ot*128+p + core_off]
Output yt [1408, 512] bf16 is transposed ([out, seq]); the host
transposes/concats (dropping the 32-row pad) while unsharding.
"""

import numpy as np
import ml_dtypes

try:
    import concourse.bass as bass
except ImportError:
    import sys

    sys.path.insert(0, "/opt/trn_rl_repo")
    import concourse.bass as bass

import concourse.mybir as mybir
import concourse.tile as tile
from concourse import bacc
from concourse.bass_utils import run_bass_kernel_spmd

B, S, IN, OUT = 4, 128, 4096, 11008
NC = 8
O_PER_CORE = OUT // NC  # 1376
O_TILES = 11  # per-core o tiles, padded: 11*128 = 1408
O_PAD = O_TILES * 128
K_TILES = IN // 128  # 32
SEQ = B * S  # 512

_E2M1_F32 = np.array(
    [0.0, 0.5, 1.0, 1.5, 2.0, 3.0, 4.0, 6.0,
     -0.0, -0.5, -1.0, -1.5, -2.0, -3.0, -4.0, -6.0],
    dtype=np.float32,
)
_LUT_FP8 = _E2M1_F32.astype(ml_dtypes.float8_e4m3).view(np.uint8)  # [16]

_COMPILED = {}


def _build_nc():
    nc = bacc.Bacc(
        "TRN2", target_bir_lowering=False, debug=False, num_devices=NC
    )
    f8 = mybir.dt.float8e4
    bf16 = mybir.dt.bfloat16
    f32 = mybir.dt.float32

    xt_d = nc.dram_tensor("xt", [128, K_TILES * SEQ], bf16, kind="ExternalInput")
    wt_d = nc.dram_tensor("wt", [O_TILES, 128, K_TILES * 128], f8, kind="ExternalInput")
    b_d = nc.dram_tensor("bias", [128, O_TILES], f32, kind="ExternalInput")
    y_d = nc.dram_tensor("yt", [O_PAD, SEQ], bf16, kind="ExternalOutput")

    from contextlib import ExitStack

    with tile.TileContext(nc) as tc, ExitStack() as ctx:
        xp = ctx.enter_context(tc.tile_pool(name="x", bufs=1))
        wp = ctx.enter_context(tc.tile_pool(name="w", bufs=1))
        pp = ctx.enter_context(tc.tile_pool(name="psum", bufs=1, space="PSUM"))
        op = ctx.enter_context(tc.tile_pool(name="out", bufs=5))
        bp = ctx.enter_context(tc.tile_pool(name="bias", bufs=1))

        W_CHUNK = 8  # kt per W DMA chunk (128 KB)

        # both HWDGE engines (Sync + Scalar) serialize their own transfers at
        # ~210 GB/s each; round-robin every input DMA across the two
        dma_rr = [0]

        def in_dma(out_ap, in_ap):
            eng = nc.sync if dma_rr[0] % 2 == 0 else nc.scalar
            dma_rr[0] += 1
            eng.dma_start(out_ap, in_ap)

        wts = {}

        def load_w_chunk(ot, j):
            if ot not in wts:
                wts[ot] = wp.tile(
                    [128, K_TILES * 128], f8, tag=f"w{ot}", name=f"w{ot}"
                )
            w_ = W_CHUNK * 128
            in_dma(
                wts[ot][:, j * w_:(j + 1) * w_], wt_d[ot, :, j * w_:(j + 1) * w_]
            )

        # PE warmup: junk matmuls (psum discarded) run during the DMA-wait
        # window at kernel start, tripping the HAM clock-gate (~3.4us busy
        # window) so real matmuls start at the warm 2.4 GHz clock.
        dj_l = xp.tile([128, 128], bf16, tag="dj_l", name="dj_l")
        dj_r = xp.tile([128, SEQ], bf16, tag="dj_r", name="dj_r")
        nc.vector.memset(dj_l[:], 0.0)
        nc.vector.memset(dj_r[:, 0:SEQ // 2], 0.0)
        nc.gpsimd.memset(dj_r[:, SEQ // 2:SEQ], 0.0)
        ps_d = pp.tile([128, SEQ], f32, tag="ps7", name="ps_d")
        for _ in range(10):
            nc.tensor.matmul(ps_d[:], lhsT=dj_l[:], rhs=dj_r[:], start=True, stop=True)

        # o-tiles processed in groups; within a group the k-loop is outer so
        # the PE consumes x chunks as they stream in.
        groups = [list(range(0, 4)), list(range(4, 8)), list(range(8, O_TILES))]

        # DMA issue order: the first matmuls' inputs go first, then W chunks
        # interleaved between x batches so everything arrives just ahead of
        # the PE (including group-1 prefetch); bias deferred (needed ~40us in)
        xts = [None] * K_TILES

        def load_x(t):
            xt = xp.tile([128, SEQ], bf16, tag=f"x{t}", name=f"x{t}")
            in_dma(xt[:], xt_d[:, t * SEQ:(t + 1) * SEQ])
            xts[t] = xt

        load_x(0)
        load_w_chunk(0, 0)
        load_w_chunk(1, 0)
        load_x(1)
        load_w_chunk(2, 0)
        load_w_chunk(3, 0)
        load_x(2)
        load_x(3)
        bt = bp.tile([128, O_TILES], f32)
        in_dma(bt[:], b_d[:])
        for t in range(4, 8):
            load_x(t)
        prefetch = {1: [(4, 0), (5, 0)], 2: [(6, 0), (7, 0)],
                    3: [(4, 1), (5, 1), (6, 1), (7, 1)]}
        for j in range(1, K_TILES // W_CHUNK):
            for ot in groups[0]:
                load_w_chunk(ot, j)
            for t in range(j * W_CHUNK, (j + 1) * W_CHUNK):
                load_x(t)
            for ot, jj in prefetch.get(j, []):
                load_w_chunk(ot, jj)
        for ot in groups[1]:
            for j in range(2, K_TILES // W_CHUNK):
                load_w_chunk(ot, j)
        for ot in groups[2]:
            for j in range(K_TILES // W_CHUNK):
                load_w_chunk(ot, j)

        for grp in groups:
            pss = {
                ot: pp.tile([128, SEQ], f32, tag=f"ps{ot % 8}", name=f"ps{ot}")
                for ot in grp
            }
            for kt in range(K_TILES):
                for ot in grp:
                    nc.tensor.matmul(
                        pss[ot][:],
                        lhsT=wts[ot][:, kt * 128:(kt + 1) * 128],
                        rhs=xts[kt][:],
                        start=(kt == 0),
                        stop=(kt == K_TILES - 1),
                    )
            for i, ot in enumerate(grp):
                # split each PSUM drain into column halves on Scalar + Vector
                ob = op.tile([128, SEQ], bf16, tag="ob", name=f"ob{ot}")
                h = SEQ // 2
                nc.scalar.activation(
                    ob[:, 0:h], pss[ot][:, 0:h],
                    mybir.ActivationFunctionType.Identity,
                    bias=bt[:, ot:ot + 1], scale=1.0,
                )
                nc.vector.tensor_scalar_add(
                    ob[:, h:SEQ], pss[ot][:, h:SEQ], bt[:, ot:ot + 1]
                )
                oeng = nc.sync if i % 2 == 0 else nc.scalar
                oeng.dma_start(y_d[ot * 128:(ot + 1) * 128, :], ob[:])

    nc.compile()
    return nc


def _prep_inputs(x, weight_packed, bias_packed):
    x = np.asarray(x)
    if x.dtype != ml_dtypes.bfloat16:
        x = x.astype(ml_dtypes.bfloat16)
    wp = np.asarray(weight_packed).astype(np.uint8)  # [OUT, IN//2]
    bp = np.asarray(bias_packed).astype(np.uint8)  # [OUT//2]

    # fp4 codes -> fp8-e4m3 bytes (exact)
    w8 = np.empty((OUT, IN), dtype=np.uint8)
    w8[:, 0::2] = _LUT_FP8[wp & 15]
    w8[:, 1::2] = _LUT_FP8[wp >> 4]

    bcodes = np.empty((OUT,), np.uint8)
    bcodes[0::2] = bp & 15
    bcodes[1::2] = bp >> 4
    bias = _E2M1_F32[bcodes]  # [OUT] f32

    # x: [B,S,IN] -> xT tiled [128, K_TILES*SEQ]
    xt = (
        np.ascontiguousarray(x.reshape(SEQ, IN).T)
        .reshape(K_TILES, 128, SEQ)
        .transpose(1, 0, 2)
        .reshape(128, K_TILES * SEQ)
    )
    xt = np.ascontiguousarray(xt)

    in_maps = []
    for c in range(NC):
        wc = w8[c * O_PER_CORE:(c + 1) * O_PER_CORE]
        wcp = np.zeros((O_PAD, IN), dtype=np.uint8)
        wcp[:O_PER_CORE] = wc
        # [ot*128+o, t*128+kk] -> [ot, kk, t*128+o]
        wt = (
            wcp.reshape(O_TILES, 128, K_TILES, 128)
            .transpose(0, 3, 2, 1)
            .reshape(O_TILES, 128, K_TILES * 128)
        )
        wt = np.ascontiguousarray(wt).view(ml_dtypes.float8_e4m3)

        bc = np.zeros((O_PAD,), dtype=np.float32)
        bc[:O_PER_CORE] = bias[c * O_PER_CORE:(c + 1) * O_PER_CORE]
        bt = np.ascontiguousarray(bc.reshape(O_TILES, 128).T)

        in_maps.append({"xt": xt, "wt": wt, "bias": bt})
    return in_maps


def _run(in_maps, **kwargs):
    if "nc" not in _COMPILED:
        _COMPILED["nc"] = _build_nc()
    return run_bass_kernel_spmd(_COMPILED["nc"], in_maps, list(range(NC)), **kwargs)


def _assemble(res):
    y = np.empty((SEQ, OUT), dtype=ml_dtypes.bfloat16)
    for c in range(NC):
        yt = np.asarray(res.results[c]["yt"])  # [O_PAD, SEQ] bf16
        y[:, c * O_PER_CORE:(c + 1) * O_PER_CORE] = yt[:O_PER_CORE].T
    return y.reshape(B, S, OUT)


def kernel(x, weight_packed, bias_packed, _bass_results=None):
    in_maps = _prep_inputs(x, weight_packed, bias_packed)
    res = _run(in_maps)
    if _bass_results is not None:
        _bass_results.append(res)
    return _assemble(res)
